# revision 1
# baseline (speedup 1.0000x reference)
"""Trainium2 Bass kernel for nn_Disc_87703232184815 (Performer discriminator).

Data-parallel over batch B=256 across 8 NeuronCores (32 examples/core).
Layout strategy per core:
  - Activations canonical FEATURE-major: hT [128(d-inner), 2(d-chunk), 4096(tokens)] bf16
  - Q/K projections feature-major (weights stationary); V + FFN2 + Wo token-major
    (activation tiles stationary) so LayerNorm / per-token ops run token-major.
  - Performer attention per (example, head):
      kp = exp(ddk) [token-major], E = exp(ddq) [feature-major],
      e^{-diag_k} folded into vext (33rd column carries ksum),
      o_ext = E^T @ (kp^T @ vext)  -> [s, 33], divide by col 32.
    KERN_EPS and max-stabilization terms are dropped (proven < 2e-6 effect).
  - All matmuls bf16 inputs / fp32 PSUM accumulation. LN internals fp32.
"""

import sys

sys.path.insert(0, "/opt/trn_rl_repo")

import numpy as np
import ml_dtypes

import concourse.bass as bass
import concourse.tile as tile
from concourse import bacc, mybir
from concourse.masks import make_identity

F32 = mybir.dt.float32
BF16 = mybir.dt.bfloat16
AX = mybir.AxisListType
OP = mybir.AluOpType
AF = mybir.ActivationFunctionType

# model constants
B, N, NDIM = 256, 127, 3
D, H, L, HID = 256, 8, 4, 256
M = HID
DH = D // H
S = N + 1            # 128 tokens incl CLS
LN_EPS = 1e-5
NORM = DH ** -0.25
RATIO = M ** -0.5    # cancels in num/den; kept out entirely

NC = 8               # cores
BC = B // NC         # 32 examples per core
T = BC * S           # 4096 tokens per core
P = 128
NKT = D // P         # 2 feature chunks
GB = 4               # examples per attention group
NG = BC // GB        # 8 groups

BF16NP = ml_dtypes.bfloat16


def _bf(x):
    return np.ascontiguousarray(x.astype(np.float32)).astype(BF16NP)


def _f32(x):
    return np.ascontiguousarray(x.astype(np.float32))


def build_nc(n_layers=L, phases='ABCEF'):
    nl = max(1, n_layers)
    nc = bacc.Bacc("TRN2", target_bir_lowering=False, debug=False, num_devices=NC)

    # ---------------- DRAM tensors ----------------
    d_xT = nc.dram_tensor("xT", (4, T), BF16, kind="ExternalInput")
    d_keep = nc.dram_tensor("keep", (P, BC), F32, kind="ExternalInput")
    d_wemb = nc.dram_tensor("wemb", (4, D), BF16, kind="ExternalInput")

    d_wq = nc.dram_tensor("wq", (nl, NKT, P, D), BF16, kind="ExternalInput")
    d_wk = nc.dram_tensor("wk", (nl, NKT, P, D), BF16, kind="ExternalInput")
    d_wvk = nc.dram_tensor("wvk", (nl, NKT, P, 2 * D), BF16, kind="ExternalInput")
    d_wo = nc.dram_tensor("wo", (nl, NKT, P, D), BF16, kind="ExternalInput")
    d_f1 = nc.dram_tensor("f1", (nl, NKT, P, D), BF16, kind="ExternalInput")
    d_f2 = nc.dram_tensor("f2", (nl, NKT, P, D), BF16, kind="ExternalInput")
    d_bq = nc.dram_tensor("bq", (nl, NKT, P), F32, kind="ExternalInput")
    d_bk = nc.dram_tensor("bk", (nl, NKT, P), F32, kind="ExternalInput")
    d_bvk = nc.dram_tensor("bvk", (nl, 2 * D), BF16, kind="ExternalInput")
    d_bo = nc.dram_tensor("bo", (nl, D), BF16, kind="ExternalInput")
    d_f1b = nc.dram_tensor("f1b", (nl, NKT, P), F32, kind="ExternalInput")
    d_f2b = nc.dram_tensor("f2b", (nl, D), BF16, kind="ExternalInput")
    d_projbq = nc.dram_tensor("projbq", (nl, 4, P, M), BF16, kind="ExternalInput")
    d_projbk = nc.dram_tensor("projbk", (nl, P, 4 * M), BF16, kind="ExternalInput")

    d_hw1 = nc.dram_tensor("hw1", (NKT, P, 2 * HID), BF16, kind="ExternalInput")
    d_hb1 = nc.dram_tensor("hb1", (4, P), F32, kind="ExternalInput")
    d_hw2 = nc.dram_tensor("hw2", (4, P, HID), BF16, kind="ExternalInput")
    d_hb2 = nc.dram_tensor("hb2", (NKT, P), F32, kind="ExternalInput")
    d_ow = nc.dram_tensor("ow", (NKT, P), BF16, kind="ExternalInput")
    d_ob = nc.dram_tensor("ob", (1, 1), F32, kind="ExternalInput")

    d_out = nc.dram_tensor("out", (BC,), F32, kind="ExternalOutput")

    with tile.TileContext(nc) as tc:
        _emit(nc, tc, n_layers, phases, locals())
    nc.compile()
    return nc


def _emit(nc, tc, n_layers, phases, d):
    from contextlib import ExitStack

    ctx = ExitStack()
    with ctx:
        # ---------------- pools ----------------
        persist = ctx.enter_context(tc.tile_pool(name="persist", bufs=1))
        wpool = ctx.enter_context(tc.tile_pool(name="wpool", bufs=2))
        sbw = ctx.enter_context(tc.tile_pool(name="sbw", bufs=3))       # working sbuf
        gwin = ctx.enter_context(tc.tile_pool(name="gwin", bufs=1))     # E/kp windows
        pp = ctx.enter_context(tc.tile_pool(name="pp", bufs=2, space="PSUM"))
        ddp = ctx.enter_context(tc.tile_pool(name="ddp", bufs=2, space="PSUM"))
        sp = ctx.enter_context(tc.tile_pool(name="sp", bufs=2, space="PSUM"))

        # ---------------- persistent tiles ----------------
        hT = persist.tile([P, NKT, T], BF16)       # canonical activations (feature-major)
        qT = persist.tile([P, NKT, T], BF16)
        kT = persist.tile([P, NKT, T], BF16)
        h1T = persist.tile([P, NKT, T], BF16)
        oT = persist.tile([P, NKT, T], BF16)
        vext = persist.tile([P, BC, H, DH + 1], BF16)
        keep_sb = persist.tile([P, BC], F32)
        diag_tok = persist.tile([P, H, BC], F32)   # sum(k^2) per (token,head,example)
        sck = persist.tile([P, H, BC], F32)        # exp(-0.5*diag)
        km = persist.tile([P, H, BC], F32)         # sck * keep
        ident = persist.tile([P, P], BF16)
        ones_row = persist.tile([1, P], BF16)      # K=1 lhsT of ones
        eps_sb = persist.tile([P, 1], F32)
        a_sb = persist.tile([P, BC, D], BF16)      # staging for LN inputs (a or f)
        mv_all = persist.tile([P, NKT, BC], F32)   # (mean, var) stash
        rstd_all = persist.tile([P, BC], F32)

        make_identity(nc, ident)
        nc.vector.memset(ones_row, 1.0)
        nc.vector.memset(eps_sb, LN_EPS)
        nc.sync.dma_start(out=keep_sb, in_=d["d_keep"].ap())

        # ---------------- embedding ----------------
        xT_sb = persist.tile([4, T], BF16, tag="xT")
        wemb_sb = persist.tile([4, D], BF16, tag="wemb")
        nc.sync.dma_start(out=xT_sb, in_=d["d_xT"].ap())
        nc.sync.dma_start(out=wemb_sb, in_=d["d_wemb"].ap())
        for c in range(NKT):
            for tci in range(8):
                ps = pp.tile([P, 512], F32, tag="pp")
                nc.tensor.matmul(ps, lhsT=wemb_sb[:, c * P:(c + 1) * P],
                                 rhs=xT_sb[:, tci * 512:(tci + 1) * 512],
                                 start=True, stop=True)
                nc.vector.tensor_copy(out=hT[:, c, tci * 512:(tci + 1) * 512], in_=ps)

        # ---------------- layers ----------------
        for l in range(n_layers):
            # ---- load weights ----
            wq_sb = wpool.tile([P, NKT, D], BF16, tag="wq")
            wk_sb = wpool.tile([P, NKT, D], BF16, tag="wk")
            wvk_sb = wpool.tile([P, NKT, 2 * D], BF16, tag="wvk")
            wo_sb = wpool.tile([P, NKT, D], BF16, tag="wo")
            f1_sb = wpool.tile([P, NKT, D], BF16, tag="f1")
            f2_sb = wpool.tile([P, NKT, D], BF16, tag="f2")
            bq_sb = wpool.tile([P, NKT], F32, tag="bq")
            bk_sb = wpool.tile([P, NKT], F32, tag="bk")
            bvk_row = wpool.tile([1, 2 * D], BF16, tag="bvk")
            bo_row = wpool.tile([1, D], BF16, tag="bo")
            f1b_sb = wpool.tile([P, NKT], F32, tag="f1b")
            f2b_row = wpool.tile([1, D], BF16, tag="f2b")
            projbq_sb = wpool.tile([P, 4, M], BF16, tag="projbq")
            projbk_sb = wpool.tile([P, 4 * M], BF16, tag="projbk")

            for t_sb, t_d in [(wq_sb, "d_wq"), (wk_sb, "d_wk"), (wvk_sb, "d_wvk"),
                              (wo_sb, "d_wo"), (f1_sb, "d_f1"), (f2_sb, "d_f2")]:
                nc.sync.dma_start(out=t_sb, in_=d[t_d].ap()[l].rearrange("k p e -> p k e"))
            nc.sync.dma_start(out=bq_sb, in_=d["d_bq"].ap()[l].rearrange("k p -> p k"))
            nc.sync.dma_start(out=bk_sb, in_=d["d_bk"].ap()[l].rearrange("k p -> p k"))
            nc.sync.dma_start(out=bvk_row, in_=d["d_bvk"].ap()[l][None, :])
            nc.sync.dma_start(out=bo_row, in_=d["d_bo"].ap()[l][None, :])
            nc.sync.dma_start(out=f1b_sb, in_=d["d_f1b"].ap()[l].rearrange("k p -> p k"))
            nc.sync.dma_start(out=f2b_row, in_=d["d_f2b"].ap()[l][None, :])
            nc.sync.dma_start(out=projbq_sb, in_=d["d_projbq"].ap()[l].rearrange("h p m -> p h m"))
            nc.sync.dma_start(out=projbk_sb, in_=d["d_projbk"].ap()[l])

            # ---- phase A: Q,K feature-major ----
            for ec in range(NKT if 'A' in phases else 0):
                for tci in range(8):
                    tsl = slice(tci * 512, (tci + 1) * 512)
                    psq = pp.tile([P, 512], F32, tag="pp")
                    for kt in range(NKT):
                        nc.tensor.matmul(psq, lhsT=wq_sb[:, kt, ec * P:(ec + 1) * P],
                                         rhs=hT[:, kt, tsl], start=(kt == 0), stop=(kt == 1))
                    nc.vector.tensor_scalar(out=qT[:, ec, tsl], in0=psq,
                                            scalar1=bq_sb[:, ec:ec + 1], scalar2=None, op0=OP.add)
                    psk = pp.tile([P, 512], F32, tag="pp")
                    for kt in range(NKT):
                        nc.tensor.matmul(psk, lhsT=wk_sb[:, kt, ec * P:(ec + 1) * P],
                                         rhs=hT[:, kt, tsl], start=(kt == 0), stop=(kt == 1))
                    nc.vector.tensor_scalar(out=kT[:, ec, tsl], in0=psk,
                                            scalar1=bk_sb[:, ec:ec + 1], scalar2=None, op0=OP.add)

            # ---- phase B: V + K-token per example ----
            for b in range(BC if 'B' in phases else 0):
                bsl = slice(b * P, (b + 1) * P)
                psvk = pp.tile([P, 512], F32, tag="pp")
                for kt in range(NKT):
                    nc.tensor.matmul(psvk, lhsT=hT[:, kt, bsl], rhs=wvk_sb[:, kt, :],
                                     start=(kt == 0), stop=False)
                nc.tensor.matmul(psvk, lhsT=ones_row, rhs=bvk_row,
                                 start=False, stop=True)
                # v part -> vext cols 0:32 per head (strided copy)
                nc.vector.tensor_copy(out=vext[:, b, :, 0:DH],
                                      in_=psvk[:, 0:D].rearrange("p (h e) -> p h e", h=H))
                # k-token part -> diag
                sq = sbw.tile([P, H, DH], F32, tag="sq")
                nc.scalar.activation(out=sq.rearrange("p h e -> p (h e)"),
                                     in_=psvk[:, D:2 * D], func=AF.Square)
                nc.vector.tensor_reduce(out=diag_tok[:, :, b], in_=sq,
                                        axis=AX.X, op=OP.add)

            if 'B' not in phases:
                continue
            # sck = exp(-0.5 * diag); km = sck * keep
            nc.scalar.activation(out=sck, in_=diag_tok, func=AF.Exp, scale=-0.5)
            nc.vector.tensor_tensor(
                out=km, in0=sck,
                in1=keep_sb[:, None, :].to_broadcast((P, H, BC)),
                op=OP.mult)

            # vext assembly (gpsimd; sbuf only, in-place scale)
            for b in range(BC):
                for h in range(H):
                    nc.gpsimd.tensor_scalar_mul(
                        out=vext[:, b, h, 0:DH],
                        in0=vext[:, b, h, 0:DH],
                        scalar1=km[:, h, b:b + 1])
                nc.gpsimd.tensor_copy(out=vext[:, b, :, DH], in_=sck[:, :, b])

            # ---- phase C: attention per group ----
            _C = (lambda c: ('C' in phases or c in phases))
            for g in range(NG if ('C' in phases or any(c in phases for c in 'qkxw')) else 0):
                # dd_q -> E (feature-major)  [P(m-inner), mc, h, 512]
                E_g = gwin.tile([P, NKT, H, GB * P], BF16, tag="Eg")
                for h in range(H if _C('q') else 0):
                    hg, hh = h // 4, h % 4
                    t0 = g * 512
                    psq = ddp.tile([P, NKT, 512], F32, tag="dd", name="psq")
                    for mc in range(NKT):
                        nc.tensor.matmul(
                            psq[:, mc],
                            lhsT=projbq_sb[:, hh, mc * P:(mc + 1) * P],
                            rhs=qT[:, hg, t0:t0 + 512],
                            start=True, stop=True)
                    nc.scalar.activation(
                        out=E_g[:, :, h, :], in_=psq, func=AF.Exp)
                # dd_k -> kp (token-major)  [P(s), b-in-g, h, m]
                kp_g = gwin.tile([P, GB, H, M], BF16, tag="kpg")
                for bb in range(GB if _C('k') else 0):
                    b = g * GB + bb
                    for hg in range(2):
                        psk = ddp.tile([P, 4, 256], F32, tag="dd", name="psk")
                        psv = psk.rearrange("p a b -> p (a b)")
                        for hf in range(2):
                            nc.tensor.matmul(
                                psv[:, hf * 512:(hf + 1) * 512],
                                lhsT=kT[:, hg, b * P:(b + 1) * P],
                                rhs=projbk_sb[:, hf * 512:(hf + 1) * 512],
                                start=True, stop=True)
                        nc.scalar.activation(
                            out=kp_g[:, bb, hg * 4:(hg + 1) * 4, :],
                            in_=psk, func=AF.Exp)

                # per-example attention
                for bb in range(GB if _C('x') else 0):
                    b = g * GB + bb
                    o_ext = sp.tile([P, H, DH + 1], F32, tag="sp")
                    for half in range(2):
                        cps = sp.tile([P, 4, NKT, DH + 1], F32, tag="sp")
                        for hh in range(4):
                            h = half * 4 + hh
                            for mc in range(NKT):
                                nc.tensor.matmul(
                                    cps[:, hh, mc],
                                    lhsT=kp_g[:, bb, h, mc * P:(mc + 1) * P],
                                    rhs=vext[:, b, h, :],
                                    start=True, stop=True)
                        csb = sbw.tile([P, 4, NKT, DH + 1], BF16, tag="csb")
                        nc.vector.tensor_copy(out=csb, in_=cps)
                        for hh in range(4):
                            h = half * 4 + hh
                            for mc in range(NKT):
                                nc.tensor.matmul(
                                    o_ext[:, h, :],
                                    lhsT=E_g[:, mc, h, bb * P:(bb + 1) * P],
                                    rhs=csb[:, hh, mc],
                                    start=(mc == 0), stop=(mc == 1))
                    dinv = sbw.tile([P, H], F32, tag="dinv")
                    nc.vector.reciprocal(dinv, o_ext[:, :, DH])
                    o_all = sbw.tile([P, H, DH], BF16, tag="oall")
                    nc.vector.tensor_tensor(
                        out=o_all, in0=o_ext[:, :, 0:DH],
                        in1=dinv[:, :, None].to_broadcast((P, H, DH)), op=OP.mult)

                    if not _C('w'):
                        continue
                    # transpose o_all -> oT
                    for c in range(NKT):
                        tp = sp.tile([P, P], BF16, tag="sp")
                        nc.tensor.transpose(
                            tp, o_all.rearrange("p h e -> p (h e)")[:, c * P:(c + 1) * P], ident)
                        nc.vector.tensor_copy(out=oT[:, c, b * P:(b + 1) * P], in_=tp)

            # ---- phase D: Wo proj + batched LN1 -> h1T ----
            for b in range(BC if 'w' in phases or 'C' in phases else 0):
                aps = sp.tile([P, D], F32, tag="sp")
                for kt in range(NKT):
                    nc.tensor.matmul(aps, lhsT=oT[:, kt, b * P:(b + 1) * P],
                                     rhs=wo_sb[:, kt, :], start=(kt == 0), stop=False)
                nc.tensor.matmul(aps, lhsT=ones_row, rhs=bo_row, start=False, stop=True)
                nc.vector.tensor_copy(out=a_sb[:, b, :], in_=aps)
                stats = sbw.tile([P, 6], F32, tag="st6")
                nc.vector.bn_stats(out=stats, in_=aps)
                nc.vector.bn_aggr(out=mv_all[:, :, b], in_=stats)
            if 'w' in phases or 'C' in phases:
                sdev_a = sbw.tile([P, BC], F32, tag="sdeva")
                nc.scalar.activation(out=sdev_a, in_=mv_all[:, 1, :], func=AF.Sqrt,
                                     bias=eps_sb)
                nc.vector.reciprocal(rstd_all, sdev_a)
                for b in range(BC):
                    h1b_tok = sbw.tile([P, D], BF16, tag="h1tok")
                    nc.vector.tensor_scalar(out=h1b_tok, in0=a_sb[:, b, :],
                                            scalar1=mv_all[:, 0, b:b + 1],
                                            scalar2=rstd_all[:, b:b + 1],
                                            op0=OP.subtract, op1=OP.mult)
                    for c in range(NKT):
                        tp = sp.tile([P, P], BF16, tag="sp")
                        nc.tensor.transpose(tp, h1b_tok[:, c * P:(c + 1) * P], ident)
                        nc.vector.tensor_copy(out=h1T[:, c, b * P:(b + 1) * P], in_=tp)

            # ---- phase E: FFN1 feature-major + leaky ----
            for hc in range(NKT if 'E' in phases else 0):
                for tci in range(8):
                    tsl = slice(tci * 512, (tci + 1) * 512)
                    ps = pp.tile([P, 512], F32, tag="pp")
                    for kt in range(NKT):
                        nc.tensor.matmul(ps, lhsT=f1_sb[:, kt, hc * P:(hc + 1) * P],
                                         rhs=h1T[:, kt, tsl], start=(kt == 0), stop=(kt == 1))
                    zt = sbw.tile([P, 512], F32, tag="zt")
                    nc.vector.tensor_scalar(out=zt, in0=ps, scalar1=f1b_sb[:, hc:hc + 1],
                                            scalar2=None, op0=OP.add)
                    nc.vector.scalar_tensor_tensor(out=qT[:, hc, tsl], in0=zt, scalar=0.2,
                                                   in1=zt, op0=OP.mult, op1=OP.max)

            # ---- phase F: FFN2 token-major + LN2 -> hT ----
            for b in range(BC if 'F' in phases else 0):
                bsl = slice(b * P, (b + 1) * P)
                fps_t = pp.tile([P, 512], F32, tag="pp", name="fps")
                fps = fps_t[:, 0:D]
                for kt in range(NKT):
                    nc.tensor.matmul(fps, lhsT=qT[:, kt, bsl], rhs=f2_sb[:, kt, :],
                                     start=(kt == 0), stop=False)
                nc.tensor.matmul(fps, lhsT=ones_row, rhs=f2b_row, start=False, stop=True)
                nc.vector.tensor_copy(out=a_sb[:, b, :], in_=fps)
                stats = sbw.tile([P, 6], F32, tag="st6")
                nc.vector.bn_stats(out=stats, in_=fps)
                nc.vector.bn_aggr(out=mv_all[:, :, b], in_=stats)
            if 'F' in phases:
                sdev_a = sbw.tile([P, BC], F32, tag="sdeva")
                nc.scalar.activation(out=sdev_a, in_=mv_all[:, 1, :], func=AF.Sqrt,
                                     bias=eps_sb)
                nc.vector.reciprocal(rstd_all, sdev_a)
                for b in range(BC):
                    h_tok = sbw.tile([P, D], BF16, tag="h1tok")
                    nc.vector.tensor_scalar(out=h_tok, in0=a_sb[:, b, :],
                                            scalar1=mv_all[:, 0, b:b + 1],
                                            scalar2=rstd_all[:, b:b + 1],
                                            op0=OP.subtract, op1=OP.mult)
                    for c in range(NKT):
                        tp = sp.tile([P, P], BF16, tag="sp")
                        nc.tensor.transpose(tp, h_tok[:, c * P:(c + 1) * P], ident)
                        nc.vector.tensor_copy(out=hT[:, c, b * P:(b + 1) * P], in_=tp)

        # ---------------- head ----------------
        hw1_sb = persist.tile([P, NKT, 2 * HID], BF16, tag="hw1")
        hb1_sb = persist.tile([P, 4], F32, tag="hb1")
        hw2_sb = persist.tile([P, 4, HID], BF16, tag="hw2")
        hb2_sb = persist.tile([P, NKT], F32, tag="hb2")
        ow_sb = persist.tile([P, NKT], BF16, tag="ows")
        ob_sb = persist.tile([1, 1], F32, tag="obs")
        nc.sync.dma_start(out=hw1_sb, in_=d["d_hw1"].ap().rearrange("k p e -> p k e"))
        nc.sync.dma_start(out=hb1_sb, in_=d["d_hb1"].ap().rearrange("k p -> p k"))
        nc.sync.dma_start(out=hw2_sb, in_=d["d_hw2"].ap().rearrange("k p e -> p k e"))
        nc.sync.dma_start(out=hb2_sb, in_=d["d_hb2"].ap().rearrange("k p -> p k"))
        nc.sync.dma_start(out=ow_sb, in_=d["d_ow"].ap().rearrange("k p -> p k"))
        nc.sync.dma_start(out=ob_sb, in_=d["d_ob"].ap())

        # CLS slice: hT[:, c, b*128] -> cT [128, NKT, BC]
        cT = hT.rearrange("p c (b s) -> p c b s", s=P)[:, :, :, 0]  # [P, NKT, BC]
        c1 = persist.tile([P, 4, BC], BF16, tag="c1")
        for oc in range(4):
            ps_t = pp.tile([P, 512], F32, tag="pp", name="hps")
            ps = ps_t[:, 0:BC]
            for kt in range(NKT):
                nc.tensor.matmul(ps, lhsT=hw1_sb[:, kt, oc * P:(oc + 1) * P],
                                 rhs=cT[:, kt, :], start=(kt == 0), stop=(kt == 1))
            t1 = sbw.tile([P, BC], F32, tag="t1")
            nc.vector.tensor_scalar(out=t1, in0=ps, scalar1=hb1_sb[:, oc:oc + 1],
                                    scalar2=None, op0=OP.add)
            nc.vector.scalar_tensor_tensor(out=c1[:, oc, :], in0=t1, scalar=0.2,
                                           in1=t1, op0=OP.mult, op1=OP.max)
        c2 = persist.tile([P, NKT, BC], BF16, tag="c2")
        for oc in range(NKT):
            ps_t = pp.tile([P, 512], F32, tag="pp", name="hps")
            ps = ps_t[:, 0:BC]
            for kt in range(4):
                nc.tensor.matmul(ps, lhsT=hw2_sb[:, kt, oc * P:(oc + 1) * P],
                                 rhs=c1[:, kt, :], start=(kt == 0), stop=(kt == 3))
            t1 = sbw.tile([P, BC], F32, tag="t1")
            nc.vector.tensor_scalar(out=t1, in0=ps, scalar1=hb2_sb[:, oc:oc + 1],
                                    scalar2=None, op0=OP.add)
            nc.vector.scalar_tensor_tensor(out=c2[:, oc, :], in0=t1, scalar=0.2,
                                           in1=t1, op0=OP.mult, op1=OP.max)
        ps_t = pp.tile([P, 512], F32, tag="pp", name="hps2")
        ps = ps_t[0:1, 0:BC]
        for kt in range(NKT):
            nc.tensor.matmul(ps, lhsT=ow_sb[:, kt:kt + 1], rhs=c2[:, kt, :],
                             start=(kt == 0), stop=(kt == 1))
        res = sbw.tile([1, BC], F32, tag="res")
        nc.vector.tensor_scalar(out=res, in0=ps, scalar1=ob_sb[0:1, 0:1], scalar2=None, op0=OP.add)
        nc.sync.dma_start(out=d["d_out"].ap()[None, :], in_=res)


# ======================================================================
# host side
# ======================================================================

def prep_inputs(inputs, n_layers=L):
    """Build the 8 per-core input maps from full-size numpy inputs."""
    dummy_pad = n_layers == 0
    if dummy_pad:
        n_layers = 1
    x = np.asarray(inputs["x"], np.float32)
    mask = np.asarray(inputs["mask"])
    g = {k: np.asarray(inputs[k], np.float32) for k in
         ["emb_w", "emb_b", "Wq", "bq", "Wk", "bk", "Wv", "bv", "Wo", "bo", "proj",
          "n1w", "n1b", "n2w", "n2b", "f1w", "f1b", "f2w", "f2b",
          "h1w", "h1b", "h2w", "h2b", "ow", "ob"]}

    # shared (weight) tensors
    shared = {}
    shared["wemb"] = _bf(np.concatenate([g["emb_w"].T, g["emb_b"][None, :]], 0))  # [4, D]

    wq_l, wk_l, wvk_l, wo_l, f1_l, f2_l = [], [], [], [], [], []
    bq_l, bk_l, bvk_l, bo_l, f1b_l, f2b_l = [], [], [], [], [], []
    projbq_l, projbk_l = [], []
    for l in range(n_layers):
        if l == 0 or dummy_pad:
            gp = np.ones(D, np.float32); bp = np.zeros(D, np.float32)
        else:
            gp = g["n2w"][l - 1]; bp = g["n2b"][l - 1]
        # q/k fold: W_eff = norm * W * gprev ; b_eff = norm*(b + W @ bprev)
        Wq_e = NORM * g["Wq"][l] * gp[None, :]
        bq_e = NORM * (g["bq"][l] + g["Wq"][l] @ bp)
        Wk_e = NORM * g["Wk"][l] * gp[None, :]
        bk_e = NORM * (g["bk"][l] + g["Wk"][l] @ bp)
        Wv_e = g["Wv"][l] * gp[None, :]
        bv_e = g["bv"][l] + g["Wv"][l] @ bp
        # feature-major lhsT layout [kt, d_inner, e_out] from W.T [d, e]
        wq_l.append(_bf(Wq_e.T.reshape(NKT, P, D)))
        wk_l.append(_bf(Wk_e.T.reshape(NKT, P, D)))
        # V + K-token combined rhs [kt, d_inner, 2D]: v cols then k cols
        wvk_l.append(_bf(np.concatenate([Wv_e.T, Wk_e.T], 1).reshape(NKT, P, 2 * D)))
        bvk_l.append(_bf(np.concatenate([bv_e, bk_e])))
        wo_l.append(_bf(g["Wo"][l].T.reshape(NKT, P, D)))
        bo_l.append(_bf(g["bo"][l]))
        # FFN fold (n1 affine into f1w)
        f1_e = g["f1w"][l] * g["n1w"][l][None, :]
        f1b_e = g["f1b"][l] + g["f1w"][l] @ g["n1b"][l]
        f1_l.append(_bf(f1_e.T.reshape(NKT, P, D)))
        f1b_l.append(_f32(f1b_e.reshape(NKT, P)))
        f2_l.append(_bf(g["f2w"][l].T.reshape(NKT, P, D)))
        f2b_l.append(_bf(g["f2b"][l]))
        bq_l.append(_f32(bq_e.reshape(NKT, P)))
        bk_l.append(_f32(bk_e.reshape(NKT, P)))
        pT = g["proj"][l].T  # [DH, M]
        pq = np.zeros((4, P, M), np.float32)
        pk = np.zeros((P, 4 * M), np.float32)
        for hh in range(4):
            pq[hh, hh * DH:(hh + 1) * DH, :] = pT
            pk[hh * DH:(hh + 1) * DH, hh * M:(hh + 1) * M] = pT
        projbq_l.append(_bf(pq)); projbk_l.append(_bf(pk))

    shared["wq"] = np.stack(wq_l); shared["wk"] = np.stack(wk_l)
    shared["wvk"] = np.stack(wvk_l); shared["wo"] = np.stack(wo_l)
    shared["f1"] = np.stack(f1_l); shared["f2"] = np.stack(f2_l)
    shared["bq"] = np.stack(bq_l); shared["bk"] = np.stack(bk_l)
    shared["bvk"] = np.stack(bvk_l); shared["bo"] = np.stack(bo_l)
    shared["f1b"] = np.stack(f1b_l); shared["f2b"] = np.stack(f2b_l)
    shared["projbq"] = np.stack(projbq_l); shared["projbk"] = np.stack(projbk_l)

    # head fold (last LN2 affine into h1w)
    if dummy_pad:
        gl = np.ones(D, np.float32); bl = np.zeros(D, np.float32)
    else:
        gl = g["n2w"][n_layers - 1]; bl = g["n2b"][n_layers - 1]
    h1w_e = g["h1w"] * gl[None, :]
    h1b_e = g["h1b"] + g["h1w"] @ bl
    shared["hw1"] = _bf(h1w_e.T.reshape(NKT, P, 2 * HID))
    shared["hb1"] = _f32(h1b_e.reshape(4, P))
    shared["hw2"] = _bf(g["h2w"].T.reshape(4, P, HID))
    shared["hb2"] = _f32(g["h2b"].reshape(NKT, P))
    shared["ow"] = _bf(g["ow"].reshape(1, D).T.reshape(NKT, P))
    shared["ob"] = _f32(g["ob"].reshape(1, 1))

    in_maps = []
    for ci in range(NC):
        xs = x[ci * BC:(ci + 1) * BC]            # [BC, N, 3]
        ms = mask[ci * BC:(ci + 1) * BC]         # [BC, N]
        xe = np.zeros((BC, S, 4), np.float32)
        xe[:, 1:, 0:3] = xs
        xe[:, 1:, 3] = 1.0
        xT = xe.reshape(BC * S, 4).T             # [4, T]
        keep = np.ones((BC, S), np.float32)
        keep[:, 1:] = (~ms).astype(np.float32)
        m = dict(shared)
        m["xT"] = _bf(xT)
        m["keep"] = _f32(keep.T)                  # [S(P), BC]
        in_maps.append(m)
    return in_maps


_NC_CACHE = {}


def kernel(**inputs):
    from concourse.bass_utils import run_bass_kernel_spmd
    n_layers = L
    if n_layers not in _NC_CACHE:
        _NC_CACHE[n_layers] = build_nc(n_layers)
    nc = _NC_CACHE[n_layers]
    in_maps = prep_inputs(inputs, n_layers)
    res = run_bass_kernel_spmd(nc, in_maps, core_ids=list(range(NC)))
    out = np.concatenate([res.results[ci]["out"] for ci in range(NC)])
    return out.reshape(B, 1).astype(np.float32)


if __name__ == "__main__":
    nc = build_nc(1)
    print("built 1-layer OK")

